# revision 4
# baseline (speedup 1.0000x reference)
"""BiosyntheticCoherenceLoss on 8 Trainium2 NeuronCores.

Scheme
------
loss needs two big reductions over the 8192x8192 pairwise-distance matrix:
  total_sum  = sum(dist)           (all pairs)
  masked_sum = sum(dist * same)    (same biosynthetic family pairs)
plus counts derivable from the codon indices alone (host).

dist is symmetric, so only the block upper-triangle is computed (weight 2 off
diagonal, 1 on diagonal).  masked_sum is computed the same way over per-family
point subsets (same-family pairs form a family x family submatrix).

Each 512x512 block computes d2[i,j] = |x_i|^2 + |x_j|^2 - 2 x_i.x_j as ONE
PSUM accumulation of two bf16 matmuls (error-compensated split):
    u = [-2x, |x|^2, 1]  (18-dim),   w = [x, 1, |x|^2]
    u = ub + du,  w = wb + dw   (bf16 value + bf16 residual)
    d2 = [ub; du; ub] . [wb; wb; dw]  =  ub.wb + du.wb + ub.dw   (one K=54 matmul)
ScalarE then does dist = Sqrt(d2 + EPS) with a free per-row accumulator
(accum_out), which is the only output (plus the padding/diagonal corrections
computed on host from a self-calibrating all-zero block).

Row padding uses u_pad = [0.. , -EPS, 0]  ->  d2 = -EPS  ->  dist exactly 0.
Col padding uses w_pad = [0.. , 0, -EPS]  ->  dist exactly 0.
pad x pad and true-diagonal elements give Sqrt(EPS) each; their exact count is
known, and the exact ACT value of Sqrt(EPS) is measured by the calibration
block (all zeros -> every element is Sqrt(EPS)).
"""
import numpy as np
import ml_dtypes

import concourse.bass as bass
from concourse import mybir
from concourse.bass_utils import run_bass_kernel_spmd

# ---------------- constants ----------------
N_CORES = 8
D = 16
K1 = 18          # [ -2x, sq, 1 ]
K2 = 54          # [ ub ; du ; ub ] vs [ wb ; wb ; dw ]
BLK = 512
EPS = 2.0 ** -8
F32 = mybir.dt.float32
BF16 = mybir.dt.bfloat16
BF = ml_dtypes.bfloat16

# fam id per codon index 0..63 (-1 = stop codon), derived from the reference's
# BIOSYNTHETIC_FAMILIES/CODON_TABLE dicts (later families overwrite on dup AA).
FAM_TABLE = np.array([
    4, 4, 3, 3, 3, 3, 3, 3, 1, 1, 1, 1, 3, 3, 3, 3,
    2, 2, 2, 2, 0, 0, 0, 0, 1, 1, 1, 1, 3, 3, 3, 3,
    4, 4, -1, -1, 5, 5, 0, 0, 1, 1, 1, 1, 1, 1, 0, 0,
    2, 2, -1, 4, 0, 0, 0, 0, 2, 2, 0, 0, 2, 2, 2, 2,
], dtype=np.int64)

_PROGRAM_CACHE: dict[int, bass.Bass] = {}


def _build_program(ntc: int) -> bass.Bass:
    """One NeuronCore program processing `ntc` 512x512 blocks."""
    if ntc in _PROGRAM_CACHE:
        return _PROGRAM_CACHE[ntc]
    NIO = 4   # input buffer depth (also the DMA semaphore lane count)
    nc = bass.Bass()
    uw = nc.declare_dram_parameter("uw", [K2, ntc * 1024], BF16, isOutput=False)
    cst = nc.declare_dram_parameter("cst", [128, 1], F32, isOutput=False)
    acc_out = nc.declare_dram_parameter("acc", [128, ntc], F32, isOutput=True)

    with (
        nc.sbuf_tensor([K2, NIO * 1024], BF16) as uw_t,
        nc.sbuf_tensor([128, 1], F32) as eps_t,
        nc.sbuf_tensor([128, ntc], F32) as acc_t,
        nc.sbuf_tensor([128, 2 * 2048], BF16) as dist_t,
        nc.psum_tensor([128, 2048], F32) as ps0,
        nc.psum_tensor([128, 2048], F32) as ps1,
        nc.semaphore() as lane0,
        nc.semaphore() as lane1,
        nc.semaphore() as lane2,
        nc.semaphore() as lane3,
        nc.semaphore() as eps_sem,
        nc.semaphore() as pe_sem,
        nc.semaphore() as act_sem,
        nc.semaphore() as dve_sem,
        nc.Block() as block,
    ):
        psums = [ps0, ps1]
        lanes = [lane0, lane1, lane2, lane3]

        # DMA lane discipline: uw block b rides lane b%NIO with target value
        # 16*(b//NIO+1).  A lane is reused only after the PE consumed the
        # previous block on it (pe_sem gate on the dma issue), so a lane's
        # count is never polluted by a still-in-flight earlier transfer.
        @block.sync
        def _(sync):
            sync.dma_start(out=eps_t[:], in_=cst[:]).then_inc(eps_sem, 16)
            for b in range(ntc):
                if b >= NIO:
                    sync.wait_ge(pe_sem, b - NIO + 1)
                j = (b % NIO) * 1024
                sync.dma_start(
                    out=uw_t[:, j:j + 1024],
                    in_=uw[:, b * 1024:(b + 1) * 1024],
                ).then_inc(lanes[b % NIO], 16)
            sync.wait_ge(dve_sem, ntc)
            sync.dma_start(out=acc_out[:], in_=acc_t[:]).then_inc(eps_sem, 16)

        @block.tensor
        def _(tensor):
            for b in range(ntc):
                tensor.wait_ge(lanes[b % NIO], 16 * (b // NIO + 1))
                if b >= 2:
                    tensor.wait_ge(act_sem, b - 1)
                base = (b % NIO) * 1024
                ps = psums[b % 2]
                for s in range(4):
                    lo, hi = base + s * 128, base + (s + 1) * 128
                    mm = nc.tensor.matmul(
                        ps[:, s * 512:(s + 1) * 512],
                        uw_t[:, lo:hi],                       # [ub ; du ; ub]
                        uw_t[:, base + 512:base + 1024],      # [wb ; wb ; dw]
                        start=True, stop=True,
                    )
                mm.then_inc(pe_sem, 1)

        @block.scalar
        def _(scalar):
            scalar.wait_ge(eps_sem, 16)
            for b in range(ntc):
                scalar.wait_ge(pe_sem, b + 1)
                if b >= 2:
                    scalar.wait_ge(dve_sem, b - 1)
                nc.scalar.activation(
                    dist_t[:, (b % 2) * 2048:(b % 2 + 1) * 2048],
                    psums[b % 2][:],
                    mybir.ActivationFunctionType.Sqrt,
                    bias=eps_t.ap(),
                ).then_inc(act_sem, 1)

        @block.vector
        def _(vector):
            for b in range(ntc):
                vector.wait_ge(act_sem, b + 1)
                nc.vector.reduce_sum(
                    acc_t[:, b:b + 1],
                    dist_t[:, (b % 2) * 2048:(b % 2 + 1) * 2048],
                    axis=mybir.AxisListType.X,
                ).then_inc(dve_sem, 1)

    _PROGRAM_CACHE[ntc] = nc
    return nc


def _prepare(codon_embeddings: np.ndarray, codon_indices: np.ndarray):
    """Host prep: build per-core packed [36, ntc*1024] bf16 inputs + metadata."""
    emb = np.ascontiguousarray(codon_embeddings, dtype=np.float32).reshape(-1, D)
    idx = np.asarray(codon_indices).reshape(-1).astype(np.int64)
    n = emb.shape[0]

    sq = np.sum(emb * emb, axis=1, dtype=np.float32)
    ones = np.ones((n, 1), np.float32)
    u = np.concatenate([-2.0 * emb, sq[:, None], ones], axis=1)   # [n, 18]
    w = np.concatenate([emb, ones, sq[:, None]], axis=1)          # [n, 18]
    ub = u.astype(BF)
    du = (u - ub.astype(np.float32)).astype(BF)
    wb = w.astype(BF)
    dw = (w - wb.astype(np.float32)).astype(BF)

    # pad sentinels: row pad -> d2 = -EPS exactly; col pad -> d2 = -EPS exactly
    u_pad = np.zeros(K1, np.float32); u_pad[16] = -EPS
    w_pad = np.zeros(K1, np.float32); w_pad[17] = -EPS
    zer = np.zeros(K1, BF)
    # K=54 packed tables: lhs = [ub ; du ; ub],  rhs = [wb ; wb ; dw]
    lhs = np.concatenate([ub, du, ub], axis=1)                    # [n, 54]
    lhs_pad = np.concatenate([u_pad.astype(BF), zer, u_pad.astype(BF)])
    rhs = np.concatenate([wb, wb, dw], axis=1)
    rhs_pad = np.concatenate([w_pad.astype(BF), w_pad.astype(BF), zer])
    lhs_all = np.concatenate([lhs, lhs_pad[None]], axis=0)        # [-1] = pad
    rhs_all = np.concatenate([rhs, rhs_pad[None]], axis=0)

    fam = FAM_TABLE[idx]
    cnt = np.bincount(fam[fam >= 0], minlength=6)

    # ---- tile list: (row_idx[512], col_idx[512], weight, cls) ----
    tiles = []
    nbA = n // BLK
    assert nbA * BLK == n
    ar = np.arange(n)
    for c in range(nbA):
        for r in range(c + 1):
            tiles.append((ar[r * BLK:(r + 1) * BLK], ar[c * BLK:(c + 1) * BLK],
                          2.0 if r < c else 1.0, 0))
    pad_sq = 0
    for f in range(6):
        mem = np.where(fam == f)[0]
        cf = len(mem)
        if cf == 0:
            continue
        nb = (cf + BLK - 1) // BLK
        padded = np.full(nb * BLK, -1, np.int64)
        padded[:cf] = mem
        pf = nb * BLK - cf
        pad_sq += pf * pf
        for j in range(nb):
            for i in range(j + 1):
                tiles.append((padded[i * BLK:(i + 1) * BLK],
                              padded[j * BLK:(j + 1) * BLK],
                              2.0 if i < j else 1.0, 1))

    # calibration block: all-zero lhs/rhs -> every element = SqrtACT(EPS)
    zero_blk = (None, None, 0.0, 2)
    tiles.append(zero_blk)
    while len(tiles) % N_CORES:
        tiles.append(zero_blk)
    ntc = len(tiles) // N_CORES

    # ---- pack per-core inputs ----
    in_maps = []
    slot_meta = []  # per core: list of (weight, cls)
    for core in range(N_CORES):
        core_tiles = tiles[core::N_CORES]
        buf = np.zeros((K2, ntc * 1024), BF)
        meta = []
        for q, (rows, cols, wgt, cls) in enumerate(core_tiles):
            o = q * 1024
            if rows is not None:
                buf[:, o:o + 512] = lhs_all[rows].T
                buf[:, o + 512:o + 1024] = rhs_all[cols].T
            meta.append((wgt, cls))
        in_maps.append({"uw": buf,
                        "cst": np.full((128, 1), EPS, np.float32)})
        slot_meta.append(meta)

    host_meta = {
        "n": n, "cnt": cnt, "pad_sq": pad_sq, "ntc": ntc,
        "slot_meta": slot_meta,
    }
    return in_maps, host_meta


def _finish(results, host_meta) -> np.float32:
    n = host_meta["n"]
    cnt = host_meta["cnt"].astype(np.float64)
    sums = [0.0, 0.0]
    cal_sum, cal_cnt = 0.0, 0
    for core, res in enumerate(results):
        acc = res["acc"].astype(np.float64)          # [128, ntc]
        ssum = acc.sum(axis=0)                       # per slot
        for q, (wgt, cls) in enumerate(host_meta["slot_meta"][core]):
            if cls == 2:
                cal_sum += ssum[q]
                cal_cnt += 1
            else:
                sums[cls] += wgt * ssum[q]
    cal = cal_sum / (cal_cnt * 128 * 2048)           # = SqrtACT(EPS)
    total_sum = sums[0] - n * cal
    masked_sum = sums[1] - (float(cnt.sum()) + host_meta["pad_sq"]) * cal

    same_count = float((cnt ** 2).sum())
    total_count = float(n) * n
    eps = 1e-10
    same_d = masked_sum / (same_count + eps)
    diff_d = (total_sum - masked_sum) / ((total_count - same_count) + eps)
    loss = same_d - 0.5 * diff_d + 1.0
    return np.float32(max(loss, 0.0))


def _run(codon_embeddings, codon_indices, trace=False):
    in_maps, host_meta = _prepare(codon_embeddings, codon_indices)
    nc = _build_program(host_meta["ntc"])
    for attempt in range(3):
        r = run_bass_kernel_spmd(nc, in_maps, list(range(N_CORES)), trace=trace)
        if all(np.isfinite(res["acc"]).all() for res in r.results):
            break
    out = _finish(r.results, host_meta)
    return out, r


def kernel(codon_embeddings, codon_indices) -> np.ndarray:
    out, _ = _run(codon_embeddings, codon_indices, trace=False)
    return np.asarray(out, dtype=np.float32)


# revision 6
# speedup vs baseline: 1.0791x; 1.0791x over previous
"""BiosyntheticCoherenceLoss on 8 Trainium2 NeuronCores.

Scheme
------
loss needs two big reductions over the 8192x8192 pairwise-distance matrix:
  total_sum  = sum(dist)           (all pairs)
  masked_sum = sum(dist * same)    (same biosynthetic family pairs)
plus counts derivable from the codon indices alone (host).

dist is symmetric, so only the block upper-triangle is computed (weight 2 off
diagonal, 1 on diagonal).  masked_sum is computed the same way over per-family
point subsets (same-family pairs form a family x family submatrix).

Each 512x512 block computes d2[i,j] = |x_i|^2 + |x_j|^2 - 2 x_i.x_j as ONE
PSUM accumulation of two bf16 matmuls (error-compensated split):
    u = [-2x, |x|^2, 1]  (18-dim),   w = [x, 1, |x|^2]
    u = ub + du,  w = wb + dw   (bf16 value + bf16 residual)
    d2 = [ub; du; ub] . [wb; wb; dw]  =  ub.wb + du.wb + ub.dw   (one K=54 matmul)
ScalarE then does dist = Sqrt(d2 + EPS) with a free per-row accumulator
(accum_out), which is the only output (plus the padding/diagonal corrections
computed on host from a self-calibrating all-zero block).

Row padding uses u_pad = [0.. , -EPS, 0]  ->  d2 = -EPS  ->  dist exactly 0.
Col padding uses w_pad = [0.. , 0, -EPS]  ->  dist exactly 0.
pad x pad and true-diagonal elements give Sqrt(EPS) each; their exact count is
known, and the exact ACT value of Sqrt(EPS) is measured by the calibration
block (all zeros -> every element is Sqrt(EPS)).
"""
import numpy as np
import ml_dtypes

import concourse.bass as bass
from concourse import mybir
from concourse.bass_utils import run_bass_kernel_spmd

# ---------------- constants ----------------
N_CORES = 8
D = 16
K1 = 18          # [ -2x, sq, 1 ]
K2 = 54          # [ ub ; du ; ub ] vs [ wb ; wb ; dw ]
BLK = 512
EPS = 2.0 ** -8
F32 = mybir.dt.float32
BF16 = mybir.dt.bfloat16
BF = ml_dtypes.bfloat16

# fam id per codon index 0..63 (-1 = stop codon), derived from the reference's
# BIOSYNTHETIC_FAMILIES/CODON_TABLE dicts (later families overwrite on dup AA).
FAM_TABLE = np.array([
    4, 4, 3, 3, 3, 3, 3, 3, 1, 1, 1, 1, 3, 3, 3, 3,
    2, 2, 2, 2, 0, 0, 0, 0, 1, 1, 1, 1, 3, 3, 3, 3,
    4, 4, -1, -1, 5, 5, 0, 0, 1, 1, 1, 1, 1, 1, 0, 0,
    2, 2, -1, 4, 0, 0, 0, 0, 2, 2, 0, 0, 2, 2, 2, 2,
], dtype=np.int64)

_PROGRAM_CACHE: dict[int, bass.Bass] = {}


def _build_program(ntc: int) -> bass.Bass:
    """One NeuronCore program processing `ntc` 512x512 blocks."""
    if ntc in _PROGRAM_CACHE:
        return _PROGRAM_CACHE[ntc]
    NIO = 4   # input buffer depth (also the DMA semaphore lane count)
    nc = bass.Bass()
    uw = nc.declare_dram_parameter("uw", [K2, ntc * 1024], BF16, isOutput=False)
    cst = nc.declare_dram_parameter("cst", [128, 1], F32, isOutput=False)
    acc_out = nc.declare_dram_parameter("acc", [128, ntc], F32, isOutput=True)

    with (
        nc.sbuf_tensor([K2, NIO * 1024], BF16) as uw_t,
        nc.sbuf_tensor([128, 1], F32) as eps_t,
        nc.sbuf_tensor([128, ntc], F32) as acc_t,
        nc.sbuf_tensor([128, 2 * 2048], BF16) as dist_t,
        nc.psum_tensor([128, 2048], F32) as ps0,
        nc.psum_tensor([128, 2048], F32) as ps1,
        nc.semaphore() as lane0,
        nc.semaphore() as lane1,
        nc.semaphore() as lane2,
        nc.semaphore() as lane3,
        nc.semaphore() as eps_sem,
        nc.semaphore() as pe_sem,
        nc.semaphore() as act_sem,
        nc.Block() as block,
    ):
        psums = [ps0, ps1]
        lanes = [lane0, lane1, lane2, lane3]

        # DMA lane discipline: uw block b rides lane b%NIO with target value
        # 16*(b//NIO+1).  A lane is reused only after the PE consumed the
        # previous block on it (pe_sem gate on the dma issue), so a lane's
        # count is never polluted by a still-in-flight earlier transfer.
        @block.sync
        def _(sync):
            for b in range(ntc):
                if b >= NIO:
                    sync.wait_ge(pe_sem, b - NIO + 1)
                j = (b % NIO) * 1024
                sync.dma_start(
                    out=uw_t[:, j:j + 1024],
                    in_=uw[:, b * 1024:(b + 1) * 1024],
                ).then_inc(lanes[b % NIO], 16)
                if b == 0:
                    sync.dma_start(out=eps_t[:], in_=cst[:]).then_inc(eps_sem, 16)
            sync.wait_ge(act_sem, ntc - 1)
            sync.dma_start(out=acc_out[:, :ntc - 1],
                           in_=acc_t[:, :ntc - 1]).then_inc(eps_sem, 16)
            sync.wait_ge(act_sem, ntc)
            with nc.allow_non_contiguous_dma(reason="single 128x1 column"):
                sync.dma_start(out=acc_out[:, ntc - 1:],
                               in_=acc_t[:, ntc - 1:]).then_inc(eps_sem, 16)

        @block.tensor
        def _(tensor):
            for b in range(ntc):
                tensor.wait_ge(lanes[b % NIO], 16 * (b // NIO + 1))
                if b >= 2:
                    tensor.wait_ge(act_sem, b - 1)
                base = (b % NIO) * 1024
                ps = psums[b % 2]
                for s in range(4):
                    lo, hi = base + s * 128, base + (s + 1) * 128
                    mm = nc.tensor.matmul(
                        ps[:, s * 512:(s + 1) * 512],
                        uw_t[:, lo:hi],                       # [ub ; du ; ub]
                        uw_t[:, base + 512:base + 1024],      # [wb ; wb ; dw]
                        start=True, stop=True,
                    )
                mm.then_inc(pe_sem, 1)

        @block.scalar
        def _(scalar):
            scalar.wait_ge(eps_sem, 16)
            for b in range(ntc):
                scalar.wait_ge(pe_sem, b + 1)
                nc.scalar.activation(
                    dist_t[:, (b % 2) * 2048:(b % 2 + 1) * 2048],
                    psums[b % 2][:],
                    mybir.ActivationFunctionType.Sqrt,
                    bias=eps_t.ap(),
                    accum_out=acc_t[:, b:b + 1],
                ).then_inc(act_sem, 1)

    _PROGRAM_CACHE[ntc] = nc
    return nc


def _prepare(codon_embeddings: np.ndarray, codon_indices: np.ndarray):
    """Host prep: build per-core packed [36, ntc*1024] bf16 inputs + metadata."""
    emb = np.ascontiguousarray(codon_embeddings, dtype=np.float32).reshape(-1, D)
    idx = np.asarray(codon_indices).reshape(-1).astype(np.int64)
    n = emb.shape[0]

    sq = np.sum(emb * emb, axis=1, dtype=np.float32)
    ones = np.ones((n, 1), np.float32)
    u = np.concatenate([-2.0 * emb, sq[:, None], ones], axis=1)   # [n, 18]
    w = np.concatenate([emb, ones, sq[:, None]], axis=1)          # [n, 18]
    ub = u.astype(BF)
    du = (u - ub.astype(np.float32)).astype(BF)
    wb = w.astype(BF)
    dw = (w - wb.astype(np.float32)).astype(BF)

    # pad sentinels: row pad -> d2 = -EPS exactly; col pad -> d2 = -EPS exactly
    u_pad = np.zeros(K1, np.float32); u_pad[16] = -EPS
    w_pad = np.zeros(K1, np.float32); w_pad[17] = -EPS
    zer = np.zeros(K1, BF)
    # K=54 packed tables: lhs = [ub ; du ; ub],  rhs = [wb ; wb ; dw]
    lhs = np.concatenate([ub, du, ub], axis=1)                    # [n, 54]
    lhs_pad = np.concatenate([u_pad.astype(BF), zer, u_pad.astype(BF)])
    rhs = np.concatenate([wb, wb, dw], axis=1)
    rhs_pad = np.concatenate([w_pad.astype(BF), w_pad.astype(BF), zer])
    lhs_all = np.concatenate([lhs, lhs_pad[None]], axis=0)        # [-1] = pad
    rhs_all = np.concatenate([rhs, rhs_pad[None]], axis=0)

    fam = FAM_TABLE[idx]
    cnt = np.bincount(fam[fam >= 0], minlength=6)

    # ---- tile list: (row_idx[512], col_idx[512], weight, cls) ----
    tiles = []
    nbA = n // BLK
    assert nbA * BLK == n
    ar = np.arange(n)
    for c in range(nbA):
        for r in range(c + 1):
            tiles.append((ar[r * BLK:(r + 1) * BLK], ar[c * BLK:(c + 1) * BLK],
                          2.0 if r < c else 1.0, 0))
    pad_sq = 0
    for f in range(6):
        mem = np.where(fam == f)[0]
        cf = len(mem)
        if cf == 0:
            continue
        nb = (cf + BLK - 1) // BLK
        padded = np.full(nb * BLK, -1, np.int64)
        padded[:cf] = mem
        pf = nb * BLK - cf
        pad_sq += pf * pf
        for j in range(nb):
            for i in range(j + 1):
                tiles.append((padded[i * BLK:(i + 1) * BLK],
                              padded[j * BLK:(j + 1) * BLK],
                              2.0 if i < j else 1.0, 1))

    # calibration block: all-zero lhs/rhs -> every element = SqrtACT(EPS)
    zero_blk = (None, None, 0.0, 2)
    tiles.append(zero_blk)
    while len(tiles) % N_CORES:
        tiles.append(zero_blk)
    ntc = len(tiles) // N_CORES

    # ---- pack per-core inputs ----
    in_maps = []
    slot_meta = []  # per core: list of (weight, cls)
    for core in range(N_CORES):
        core_tiles = tiles[core::N_CORES]
        buf = np.zeros((K2, ntc * 1024), BF)
        meta = []
        for q, (rows, cols, wgt, cls) in enumerate(core_tiles):
            o = q * 1024
            if rows is not None:
                buf[:, o:o + 512] = lhs_all[rows].T
                buf[:, o + 512:o + 1024] = rhs_all[cols].T
            meta.append((wgt, cls))
        in_maps.append({"uw": buf,
                        "cst": np.full((128, 1), EPS, np.float32)})
        slot_meta.append(meta)

    host_meta = {
        "n": n, "cnt": cnt, "pad_sq": pad_sq, "ntc": ntc,
        "slot_meta": slot_meta,
    }
    return in_maps, host_meta


def _finish(results, host_meta) -> np.float32:
    n = host_meta["n"]
    cnt = host_meta["cnt"].astype(np.float64)
    sums = [0.0, 0.0]
    cal_sum, cal_cnt = 0.0, 0
    for core, res in enumerate(results):
        acc = res["acc"].astype(np.float64)          # [128, ntc]
        ssum = acc.sum(axis=0)                       # per slot
        for q, (wgt, cls) in enumerate(host_meta["slot_meta"][core]):
            if cls == 2:
                cal_sum += ssum[q]
                cal_cnt += 1
            else:
                sums[cls] += wgt * ssum[q]
    cal = cal_sum / (cal_cnt * 128 * 2048)           # = SqrtACT(EPS)
    total_sum = sums[0] - n * cal
    masked_sum = sums[1] - (float(cnt.sum()) + host_meta["pad_sq"]) * cal

    same_count = float((cnt ** 2).sum())
    total_count = float(n) * n
    eps = 1e-10
    same_d = masked_sum / (same_count + eps)
    diff_d = (total_sum - masked_sum) / ((total_count - same_count) + eps)
    loss = same_d - 0.5 * diff_d + 1.0
    return np.float32(max(loss, 0.0))


def _run(codon_embeddings, codon_indices, trace=False):
    in_maps, host_meta = _prepare(codon_embeddings, codon_indices)
    nc = _build_program(host_meta["ntc"])
    for attempt in range(3):
        r = run_bass_kernel_spmd(nc, in_maps, list(range(N_CORES)), trace=trace)
        if all(np.isfinite(res["acc"]).all() for res in r.results):
            break
    out = _finish(r.results, host_meta)
    return out, r


def kernel(codon_embeddings, codon_indices) -> np.ndarray:
    out, _ = _run(codon_embeddings, codon_indices, trace=False)
    return np.asarray(out, dtype=np.float32)
